# revision 12
# baseline (speedup 1.0000x reference)
"""DeepseekV4-style MoE block on 8 trn2 NeuronCores.

Sharding: expert-parallel. Each core owns E/8 = 2 experts (dense compute over
all tokens; router weights make unselected contributions zero), plus a
1/8 slice of the shared-expert intermediate dim. Partial [H, T] outputs are
summed with on-device ReduceScatters (one per token half, so the first
overlaps the second half's compute); core c returns rows 128c..128(c+1)
of y^T. The host concatenates and transposes back to [T, H].

Layouts are transposed ([feature, token]) so no on-device transposes of
activations are needed:
  gT[i,t] = sum_h W1[h,i] xT[h,t]      (lhsT = W1 tile, rhs = xT tile)
  yT[h,t] = sum_i W2[i,h] actw[i,t]    (lhsT = W2 tile, rhs = actw tile)
Router runs in [token, expert] layout (fp32 matmuls for exact top-k), then a
PE transpose + one-hot selection matmul broadcasts each local expert's
per-token weight across 128 partitions.
"""

import sys

sys.path.insert(0, "/opt/trn_rl_repo")

import numpy as np
import ml_dtypes

import concourse.bass as bass
import concourse.mybir as mybir
import concourse.tile as tile
from concourse import bacc
from concourse.masks import make_identity

T, H, E, I, K = 1024, 1024, 16, 512, 4
IS = 1024
NCORES = 8
EPC = E // NCORES          # experts per core = 2
ISC = IS // NCORES         # shared-intermediate slice = 128
LIMIT = 7.0
SCALE = 2.5
P = 128
KT = H // P                # 8 contraction tiles
TT = T // P                # 8 token tiles
NT = 512                   # matmul moving free-dim chunk
THn = T // NT              # 2 token halves
IT = I // P                # 4 i-tiles per expert
HT = H // P                # 8 h-tiles

f32 = mybir.dt.float32
bf16 = mybir.dt.bfloat16
AF = mybir.ActivationFunctionType
ALU = mybir.AluOpType
AX = mybir.AxisListType

NEG = -1.0e30


def declare_io(nc):
    io = {}
    io["xt_f"] = nc.dram_tensor("xt_f", [KT, P, T], f32, kind="ExternalInput")
    io["xt_b"] = nc.dram_tensor("xt_b", [KT, P, T], bf16, kind="ExternalInput")
    io["wgt"] = nc.dram_tensor("wgt", [KT, P, E], f32, kind="ExternalInput")
    io["biasr"] = nc.dram_tensor("biasr", [1, E], f32, kind="ExternalInput")
    io["selm"] = nc.dram_tensor("selm", [E, EPC * P], f32, kind="ExternalInput")
    io["w1t"] = nc.dram_tensor("w1t", [EPC * KT, P, I], bf16, kind="ExternalInput")
    io["w3t"] = nc.dram_tensor("w3t", [EPC * KT, P, I], bf16, kind="ExternalInput")
    io["w2t"] = nc.dram_tensor("w2t", [EPC * IT, P, H], bf16, kind="ExternalInput")
    io["wsgt"] = nc.dram_tensor("wsgt", [KT, P, ISC], bf16, kind="ExternalInput")
    io["wsut"] = nc.dram_tensor("wsut", [KT, P, ISC], bf16, kind="ExternalInput")
    io["wsdt"] = nc.dram_tensor("wsdt", [ISC, H], bf16, kind="ExternalInput")
    io["out"] = nc.dram_tensor("out", [P, T], f32, kind="ExternalOutput")
    return io


def emit_body(nc, pools, io, rs_in, rs_out, with_collective=True):
    consts, wpool, rwork, awork, psum = pools
    xt_f, xt_b, wgt_d, biasr, selm = (
        io["xt_f"], io["xt_b"], io["wgt"], io["biasr"], io["selm"]
    )
    w1t, w3t, w2t = io["w1t"], io["w3t"], io["w2t"]
    wsgt, wsut, wsdt, out = io["wsgt"], io["wsut"], io["wsdt"], io["out"]

    # ---------------- constant / weight loads ----------------
    ident = consts.tile([P, P], f32, name="ident")
    make_identity(nc, ident)

    # bf16 x first: expert compute must not wait for the router's fp32 copy
    xtb = []
    for k in range(KT):
        t2 = consts.tile([P, T], bf16, name=f"xtb{k}")
        nc.sync.dma_start(t2, xt_b[k])
        xtb.append(t2)

    w1, w3, w2 = [], [], []
    for j in range(EPC):
        for k in range(KT):
            tw = wpool.tile([P, I], bf16, name=f"w1_{j}_{k}")
            nc.sync.dma_start(tw, w1t[j * KT + k])
            w1.append(tw)
        for k in range(KT):
            tw = wpool.tile([P, I], bf16, name=f"w3_{j}_{k}")
            nc.sync.dma_start(tw, w3t[j * KT + k])
            w3.append(tw)

    xtf, wg = [], []
    for k in range(KT):
        t1 = consts.tile([P, T], f32, name=f"xtf{k}")
        nc.sync.dma_start(t1, xt_f[k])
        xtf.append(t1)
        t3 = consts.tile([P, E], f32, name=f"wg{k}")
        nc.sync.dma_start(t3, wgt_d[k])
        wg.append(t3)

    bias_bc = consts.tile([P, E], f32, name="bias_bc")
    nc.sync.dma_start(bias_bc, biasr[:].to_broadcast((P, E)))
    selm_sb = consts.tile([E, EPC * P], f32, name="selm_sb")
    nc.sync.dma_start(selm_sb, selm[:])

    wsg, wsu = [], []
    for k in range(KT):
        tw = wpool.tile([P, ISC], bf16, name=f"wsg{k}")
        nc.sync.dma_start(tw, wsgt[k])
        wsg.append(tw)
        tw2 = wpool.tile([P, ISC], bf16, name=f"wsu{k}")
        nc.sync.dma_start(tw2, wsut[k])
        wsu.append(tw2)
    for j in range(EPC):
        for i in range(IT):
            tw = wpool.tile([P, H], bf16, name=f"w2_{j}_{i}")
            nc.sync.dma_start(tw, w2t[j * IT + i])
            w2.append(tw)
    wsd_sb = wpool.tile([ISC, H], bf16, name="wsd_sb")
    nc.sync.dma_start(wsd_sb, wsdt[:])

    # ---------------- router ----------------
    # logits[t,e] (fp32, exact) -> scores = sqrt(log1p(exp(.)))
    lscs = []
    for tt in range(TT):
        lg = psum.tile([P, E], f32, name="lg", tag="lg", bufs=2)
        for k in range(KT):
            nc.tensor.matmul(
                lg,
                lhsT=xtf[k][:, tt * P:(tt + 1) * P],
                rhs=wg[k],
                start=(k == 0),
                stop=(k == KT - 1),
            )
        esc = rwork.tile([P, E], f32, name="esc", tag="esc")
        nc.scalar.activation(esc, lg, AF.Exp)
        lsc = rwork.tile([P, E], f32, name=f"lsc{tt}", tag=f"lsc{tt}", bufs=1)
        nc.scalar.activation(lsc, esc, AF.Ln, bias=1.0)
        lscs.append(lsc)
    scrs = []
    for tt in range(TT):
        scr = rwork.tile([P, E], f32, name=f"scr{tt}", tag=f"scr{tt}", bufs=1)
        nc.scalar.activation(scr, lscs[tt], AF.Sqrt)
        scrs.append(scr)

    w_dT = consts.tile([E, T], f32, name="w_dT")
    for tt in range(TT):
        scr = scrs[tt]
        sb = rwork.tile([P, E], f32, name="sb", tag="sb")
        nc.vector.tensor_add(sb, scr, bias_bc)
        mx8 = rwork.tile([P, 8], f32, name="mx8", tag="mx8")
        nc.vector.max(out=mx8, in_=sb)
        nc.vector.memset(mx8[:, K:8], NEG)
        rep = rwork.tile([P, E], f32, name="rep", tag="rep")
        nc.vector.match_replace(
            out=rep, in_to_replace=mx8, in_values=sb, imm_value=NEG
        )
        msk = rwork.tile([P, E], f32, name="msk", tag="msk")
        nc.vector.tensor_tensor(msk, sb, rep, op=ALU.not_equal)
        wsel = rwork.tile([P, E], f32, name="wsel", tag="wsel")
        nc.vector.tensor_mul(wsel, msk, scr)
        den = rwork.tile([P, 1], f32, name="den", tag="den")
        nc.vector.reduce_sum(den, wsel, axis=AX.X)
        rin = rwork.tile([P, 1], f32, name="rin", tag="rin")
        nc.vector.reciprocal(rin, den)
        wd = rwork.tile([P, E], f32, name="wd", tag="wd")
        nc.vector.tensor_scalar(wd, wsel, rin, float(SCALE), ALU.mult, ALU.mult)
        wtp = psum.tile([E, P], f32, name="wtp", tag="wtp", bufs=2)
        nc.tensor.transpose(wtp, wd, ident)
        nc.vector.tensor_copy(w_dT[:, tt * P:(tt + 1) * P], wtp)

    # broadcast local experts' per-token weights across partitions:
    # wb_j[p, t] = w_dT[2c+j, t] via one-hot selection matmul
    wb = []
    for j in range(EPC):
        wbt = consts.tile([P, T], f32, name=f"wb{j}")
        for th in range(THn):
            wps = psum.tile([P, NT], f32, name="wps", tag="mm", bufs=4)
            nc.tensor.matmul(
                wps,
                lhsT=selm_sb[:, j * P:(j + 1) * P],
                rhs=w_dT[:, th * NT:(th + 1) * NT],
                start=True,
                stop=True,
            )
            nc.vector.tensor_copy(wbt[:, th * NT:(th + 1) * NT], wps)
        wb.append(wbt)

    # ------------- per token-half: experts, shared, down, RS -------
    for th in range(THn):
        tsl = slice(th * NT, (th + 1) * NT)
        aw = []
        for j in range(EPC):
            for i in range(IT):
                gps = psum.tile([P, NT], f32, name="gps", tag="mm", bufs=4)
                for k in range(KT):
                    nc.tensor.matmul(
                        gps,
                        lhsT=w1[j * KT + k][:, i * P:(i + 1) * P],
                        rhs=xtb[k][:, tsl],
                        start=(k == 0),
                        stop=(k == KT - 1),
                    )
                g_sb = awork.tile([P, NT], bf16, name="g_sb", tag="g_sb")
                nc.vector.tensor_scalar_min(g_sb, gps, LIMIT)
                ups = psum.tile([P, NT], f32, name="ups", tag="mm", bufs=4)
                for k in range(KT):
                    nc.tensor.matmul(
                        ups,
                        lhsT=w3[j * KT + k][:, i * P:(i + 1) * P],
                        rhs=xtb[k][:, tsl],
                        start=(k == 0),
                        stop=(k == KT - 1),
                    )
                u_sb = awork.tile([P, NT], bf16, name="u_sb", tag="u_sb")
                nc.vector.tensor_scalar(
                    u_sb, ups, LIMIT, -LIMIT, ALU.min, ALU.max
                )
                sg = awork.tile([P, NT], bf16, name="sg", tag="sg")
                nc.scalar.activation(sg, g_sb, AF.Sigmoid, scale=1.702)
                awt = awork.tile([P, NT], bf16, name=f"aw{th}_{j}_{i}", bufs=1)
                # awt = (u+1) * sigmoid(1.702 g) * g * w
                nc.vector.scalar_tensor_tensor(
                    awt, in0=u_sb, scalar=1.0, in1=sg,
                    op0=ALU.add, op1=ALU.mult,
                )
                nc.vector.tensor_mul(awt, awt, g_sb)
                nc.vector.tensor_mul(awt, awt, wb[j][:, tsl])
                aw.append(awt)

        # shared expert slice
        sgp = psum.tile([P, NT], f32, name="sgp", tag="mm", bufs=4)
        for k in range(KT):
            nc.tensor.matmul(
                sgp, lhsT=wsg[k], rhs=xtb[k][:, tsl],
                start=(k == 0), stop=(k == KT - 1),
            )
        sup = psum.tile([P, NT], f32, name="sup", tag="mm", bufs=4)
        for k in range(KT):
            nc.tensor.matmul(
                sup, lhsT=wsu[k], rhs=xtb[k][:, tsl],
                start=(k == 0), stop=(k == KT - 1),
            )
        sigs = awork.tile([P, NT], bf16, name="sigs", tag="sigs")
        nc.scalar.activation(sigs, sgp, AF.Sigmoid)
        gsb = awork.tile([P, NT], bf16, name="gsb", tag="gsb")
        nc.vector.tensor_copy(gsb, sgp)
        # only one PSUM operand allowed per DVE instruction
        usig = awork.tile([P, NT], f32, name="usig", tag="usig")
        nc.vector.tensor_mul(usig, sup, sigs)
        s_sb = awork.tile([P, NT], bf16, name="s_sb", tag="s_sb")
        nc.vector.tensor_mul(s_sb, usig, gsb)

        # down projection
        for h in range(HT):
            yps = psum.tile([P, NT], f32, name="yps", tag="mm", bufs=4)
            nmm = EPC * IT + 1
            idx = 0
            for j in range(EPC):
                for i in range(IT):
                    nc.tensor.matmul(
                        yps,
                        lhsT=w2[j * IT + i][:, h * P:(h + 1) * P],
                        rhs=aw[j * IT + i],
                        start=(idx == 0),
                        stop=(idx == nmm - 1),
                    )
                    idx += 1
            nc.tensor.matmul(
                yps,
                lhsT=wsd_sb[:, h * P:(h + 1) * P],
                rhs=s_sb,
                start=False,
                stop=True,
            )
            yt = awork.tile([P, NT], f32, name="yt", tag="yt")
            nc.vector.tensor_copy(yt, yps)
            nc.sync.dma_start(rs_in[th][h * P:(h + 1) * P, :], yt)

        if with_collective:
            nc.gpsimd.collective_compute(
                "ReduceScatter",
                ALU.add,
                replica_groups=[list(range(NCORES))],
                ins=[rs_in[th].opt()],
                outs=[rs_out[th].opt()],
            )
            nc.sync.dma_start(out[:, tsl], rs_out[th][:])
        else:
            nc.sync.dma_start(out[:, tsl], rs_in[th][0:P, :])


def build_nc(with_collective=True, bench_loop=0):
    nc = bacc.Bacc(None, num_devices=NCORES)
    io = declare_io(nc)

    with tile.TileContext(nc) as tc:
        with (
            tc.tile_pool(name="consts", bufs=1) as consts,
            tc.tile_pool(name="wpool", bufs=1) as wpool,
            tc.tile_pool(name="rwork", bufs=2) as rwork,
            tc.tile_pool(name="awork", bufs=2) as awork,
            tc.tile_pool(name="psum", bufs=1, space="PSUM") as psum,
            tc.tile_pool(name="dram", bufs=1, space="DRAM") as dram,
        ):
            rs_in = [dram.tile([H, NT], f32, name=f"rs_in{th}") for th in range(THn)]
            rs_out = [dram.tile([P, NT], f32, name=f"rs_out{th}") for th in range(THn)]
            pools = (consts, wpool, rwork, awork, psum)
            if bench_loop:
                with tc.For_i(0, bench_loop, 1):
                    emit_body(nc, pools, io, rs_in, rs_out, with_collective=False)
            else:
                emit_body(nc, pools, io, rs_in, rs_out, with_collective)

    nc.compile()
    return nc


def make_in_maps(inputs):
    x = np.asarray(inputs["hidden_states"], np.float32)
    Wg = np.asarray(inputs["Wg"], np.float32)
    bias = np.asarray(inputs["bias"], np.float32)
    W1 = np.asarray(inputs["W1"], np.float32)
    W3 = np.asarray(inputs["W3"], np.float32)
    W2 = np.asarray(inputs["W2"], np.float32)
    Wsg = np.asarray(inputs["Wsg"], np.float32)
    Wsu = np.asarray(inputs["Wsu"], np.float32)
    Wsd = np.asarray(inputs["Wsd"], np.float32)

    bf = ml_dtypes.bfloat16
    xT = np.ascontiguousarray(x.T)                       # [H, T]
    xt_f = xT.reshape(KT, P, T)
    xt_b = xt_f.astype(bf)
    wgt = np.ascontiguousarray(Wg.T).reshape(KT, P, E)
    biasr = bias.reshape(1, E).copy()
    W1b = W1.astype(bf)
    W3b = W3.astype(bf)
    W2b = W2.astype(bf)
    Wsgb = Wsg.astype(bf)
    Wsub = Wsu.astype(bf)
    Wsdb = Wsd.astype(bf)

    in_maps = []
    for c in range(NCORES):
        sel = np.zeros((E, EPC * P), np.float32)
        for j in range(EPC):
            sel[c * EPC + j, j * P:(j + 1) * P] = 1.0
        w1c = np.ascontiguousarray(W1b[c * EPC:(c + 1) * EPC].reshape(EPC * KT, P, I))
        w3c = np.ascontiguousarray(W3b[c * EPC:(c + 1) * EPC].reshape(EPC * KT, P, I))
        w2c = np.ascontiguousarray(W2b[c * EPC:(c + 1) * EPC].reshape(EPC * IT, P, H))
        wsgc = np.ascontiguousarray(Wsgb[:, c * ISC:(c + 1) * ISC]).reshape(KT, P, ISC)
        wsuc = np.ascontiguousarray(Wsub[:, c * ISC:(c + 1) * ISC]).reshape(KT, P, ISC)
        wsdc = np.ascontiguousarray(Wsdb[c * ISC:(c + 1) * ISC, :])
        in_maps.append(
            {
                "xt_f": xt_f,
                "xt_b": xt_b,
                "wgt": wgt,
                "biasr": biasr,
                "selm": sel,
                "w1t": w1c,
                "w3t": w3c,
                "w2t": w2c,
                "wsgt": wsgc,
                "wsut": wsuc,
                "wsdt": wsdc,
            }
        )
    return in_maps


def assemble(per_core_outs):
    ytT = np.concatenate([np.asarray(o) for o in per_core_outs], axis=0)  # [H, T]
    return np.ascontiguousarray(ytT.T.astype(np.float32))


_NC_CACHE = []

TRACE = False


def kernel(**inputs):
    from concourse.bass_utils import run_bass_kernel_spmd

    if not _NC_CACHE:
        _NC_CACHE.append(build_nc())
    nc = _NC_CACHE[0]
    in_maps = make_in_maps(inputs)
    res = run_bass_kernel_spmd(
        nc, in_maps, core_ids=list(range(NCORES)), trace=TRACE
    )
    if TRACE:
        kernel.last_results = res
    return assemble([res.results[c]["out"] for c in range(NCORES)])
